# revision 9
# baseline (speedup 1.0000x reference)
"""BrainGCN Trainium2 kernel (8 NeuronCores, Bass/Tile).

Model (PyG-style GCNConv x2 + 2 FC layers):
    h = tanh(gcn(x,  W1, b1)); h = tanh(gcn(h, W2, b2))
    h = tanh(h @ W3 + b3);      out = h @ W4 + b4

gcn(x, W, b) = (agg + x * dinv^2) @ W + b  with
    agg[d] = sum_{e:(s,d)} dinv[s]*dinv[d] * x[s]        (by linearity we
aggregate raw feature rows first, then apply W once per node).

Distribution: dst-nodes are split into 8 contiguous blocks (one per core).
Each core aggregates its own dst block; self-loops are folded in via a
host-built diagonal S block.  The only cross-core exchange is an AllGather
of the h1 shards between the two GCN layers.

Edge slots are laid out in two half-major regions (src < / >= HALF so the
int16 gather indices stay in range), window-major inside each region.  The
regions are gathered by uniform 1024-slot dma_gather calls (single_packet
concat chains cap out at 64 descriptors = 16KB per SDMA lane) that rotate
over the 4 SWDGE queues; windows then accumulate tiles from both regions
into one PSUM bank, so no extra pass is needed.  For every 128-slot tile
the PE accumulates aggT[feat, dst] += E_tile^T @ S into PSUM where S is a
host-built coef-scattered block streamed from HBM.
"""

import numpy as np

# ---------------------------------------------------------------- constants
N_NODES = 50000
N_CORES = 8
F_IN, H1D, H2D, H3D, OUTD = 128, 128, 64, 64, 1
WIN = 128          # dst window width (psum free dim of the scatter matmul)
HALF = 25000       # gather-table region size (int16 index range)
CALL = 1024        # gather slots per dma_gather call (64 descs per lane)
NCHUNK = 512       # fc-layer column chunk


def _cdiv(a, b):
    return -(-a // b)


def _rup(a, b):
    return _cdiv(a, b) * b


# ------------------------------------------------------------------ planning
class Plan:
    pass


def make_plan(edge_index, n_nodes=N_NODES, n_cores=N_CORES, half=HALF,
              win=WIN, call_sz=CALL):
    """Host-side graph preprocessing -> static schedule + per-core arrays."""
    src = np.asarray(edge_index[0]).astype(np.int64)
    dst = np.asarray(edge_index[1]).astype(np.int64)

    npc = n_nodes // n_cores
    assert npc * n_cores == n_nodes
    n_win = _cdiv(npc, win)
    n_half = _cdiv(n_nodes, half)
    assert half <= 32767

    deg = np.bincount(dst, minlength=n_nodes).astype(np.float64) + 1.0
    dinv = 1.0 / np.sqrt(deg)

    s_all, d_all = src, dst
    coef = (dinv[s_all] * dinv[d_all]).astype(np.float32)

    core = d_all // npc
    w = (d_all % npc) // win
    h = s_all // half

    # per (core, w, h) counts -> static caps shared by all cores (SPMD)
    gid = (core * n_win + w) * n_half + h
    counts = np.bincount(gid, minlength=n_cores * n_win * n_half)
    caps = counts.reshape(n_cores, n_win, n_half).max(axis=0)  # [n_win,n_half]

    # ---- half-major slot regions, window-major inside
    off = np.zeros((n_win, n_half), dtype=np.int64)
    regions = []                  # (base, real_end, padded_end) per half
    pos = 0
    for hh in range(n_half):
        base = pos
        for ww in range(n_win):
            off[ww, hh] = pos
            pos += int(caps[ww, hh])
        real_end = pos
        pos = _rup(pos, 128)
        regions.append((base, real_end, pos))
    S = pos
    T = S // 128

    # ---- uniform gather calls per region
    calls = []                    # (h, slot_off, n_slots)
    calls_h = [[] for _ in range(n_half)]
    for hh, (base, _re, pend) in enumerate(regions):
        o = base
        while o < pend:
            n = min(call_sz, pend - o)
            calls_h[hh].append(len(calls))
            calls.append((hh, o, n))
            o += n
    call_of_tile = {}             # global tile -> (call_global, tile_in_call)
    for ci, (hh, o, n) in enumerate(calls):
        for t in range(n // 128):
            call_of_tile[o // 128 + t] = (ci, t)
    local_of_call = {}            # global call -> region-local index
    for hh in range(n_half):
        for li, ci in enumerate(calls_h[hh]):
            local_of_call[ci] = li

    # ---- per-window tile/ent lists in processing order
    ents = []                     # (ww, hh, ci, tin, t_global)
    win_ents = []                 # per window: (q0, [(hh, ci, tin)...])
    win_need = []                 # per window: max region-local call idx per h
    for ww in range(n_win):
        lst = []
        need = [-1] * n_half
        for hh in range(n_half):
            o, cp = int(off[ww, hh]), int(caps[ww, hh])
            if cp == 0:
                continue
            for t in range(o // 128, (o + cp - 1) // 128 + 1):
                ci, tin = call_of_tile[t]
                lst.append((hh, ci, tin))
                need[hh] = max(need[hh], local_of_call[ci])
                ents.append((ww, hh, ci, tin, t))
        win_ents.append((len(ents) - len(lst), lst))
        win_need.append(need)
    NE = len(ents)

    # ---- per-core arrays: order edges by (core, h, w), rank within group
    order_key = (core * n_half + h) * n_win + w
    order = np.argsort(order_key, kind="stable")
    s_o, d_o, c_o = s_all[order], d_all[order], coef[order]
    core_o, w_o, h_o = core[order], w[order], h[order]

    comb = (core_o * n_half + h_o) * n_win + w_o
    pos_a = np.arange(comb.size, dtype=np.int64)
    is_start = np.ones(comb.size, dtype=bool)
    if comb.size > 1:
        is_start[1:] = comb[1:] != comb[:-1]
    rank = pos_a - np.maximum.accumulate(np.where(is_start, pos_a, 0))
    slot = off[w_o, h_o] + rank

    idx16 = np.zeros((n_cores, S), dtype=np.int16)
    dloc = np.zeros((n_cores, S), dtype=np.int64)
    cof = np.zeros((n_cores, S), dtype=np.float32)
    ci_core = core_o.astype(np.int64)
    idx16[ci_core, slot] = (s_o - h_o * half).astype(np.int16)
    dloc[ci_core, slot] = d_o - ci_core * npc - w_o * win
    cof[ci_core, slot] = c_o
    for (_b, real_end, pend) in regions:
        idx16[:, real_end:pend] = -1       # trailing pads: ucode trims them

    p = Plan()
    p.n_nodes, p.n_cores, p.npc = n_nodes, n_cores, npc
    p.win, p.n_win, p.half, p.n_half = win, n_win, half, n_half
    p.win_sizes = [min(win, npc - ww * win) for ww in range(n_win)]
    p.calls, p.calls_h = calls, calls_h
    p.win_ents, p.win_need = win_ents, win_need
    p.S, p.T, p.NE = S, T, NE
    # idx layout for the device: [128, S/16] (16-row wrap, replicated x8)
    p.idx_arr = np.ascontiguousarray(
        np.tile(idx16.reshape(n_cores, S // 16, 16).transpose(0, 2, 1),
                (1, 8, 1)))

    # window owner of each static slot (pad slots: -1)
    w_of_slot = np.full(S, -1, dtype=np.int64)
    for ww in range(n_win):
        for hh in range(n_half):
            o = int(off[ww, hh])
            w_of_slot[o:o + int(caps[ww, hh])] = ww

    # host-built S tiles in ents order:
    # smat[c, p, q*win + dloc] = coef if slot t*128+p belongs to window ww
    smat = np.zeros((n_cores, 128, NE * win), dtype=np.float16)
    cidx = np.arange(n_cores)[:, None]
    for q, (ww, hh, ci, tin, t) in enumerate(ents):
        sl = np.arange(t * 128, t * 128 + 128)
        m = w_of_slot[sl] == ww
        rows = np.where(m)[0]
        if rows.size == 0:
            continue
        smat[cidx, rows[None, :], q * win + dloc[:, sl[m]]] = cof[:, sl[m]]
    p.smat = np.ascontiguousarray(smat)

    # self-loop diagonal: sdiag[c, p, w*win + j] = (p==j)*dinv^2[global node]
    d2 = (dinv * dinv).astype(np.float32)
    sdiag = np.zeros((n_cores, 128, n_win * win), dtype=np.float16)
    for c in range(n_cores):
        for ww in range(n_win):
            wsz = min(win, npc - ww * win)
            g0 = c * npc + ww * win
            sdiag[c, np.arange(wsz), ww * win + np.arange(wsz)] = d2[g0:g0 + wsz]
    p.sdiag = sdiag

    return p


# ------------------------------------------------------------------- program
def build_program(p, debug=False, n_queues=4, scratch=32768,
                  edge_dt="float16"):
    import concourse.bacc as bacc
    import concourse.mybir as mybir
    import concourse.tile as tile
    from concourse.masks import make_identity

    f32 = mybir.dt.float32
    edt = getattr(mybir.dt, edge_dt)
    i16 = mybir.dt.int16
    AF = mybir.ActivationFunctionType
    OP = mybir.AluOpType

    nc = bacc.Bacc("TRN2", target_bir_lowering=False, debug=debug,
                   num_devices=p.n_cores, num_swdge_queues=n_queues,
                   dynamic_dma_scratch_size=scratch)

    x_d = nc.dram_tensor("x", [p.n_nodes, F_IN], edt, kind="ExternalInput")
    idx_d = nc.dram_tensor("midx", [128, p.S // 16], i16, kind="ExternalInput")
    smat_d = nc.dram_tensor("smat", [128, p.NE * p.win], edt,
                            kind="ExternalInput")
    sdiag_d = nc.dram_tensor("sdiag", [128, p.n_win * p.win], edt,
                             kind="ExternalInput")
    xown_d = nc.dram_tensor("xown", [p.npc, F_IN], edt, kind="ExternalInput")
    w1_d = nc.dram_tensor("w1", [F_IN, H1D], f32, kind="ExternalInput")
    b1_d = nc.dram_tensor("b1", [H1D, 1], f32, kind="ExternalInput")
    w2_d = nc.dram_tensor("w2", [H1D, H2D], f32, kind="ExternalInput")
    b2_d = nc.dram_tensor("b2", [H2D, 1], f32, kind="ExternalInput")
    w3_d = nc.dram_tensor("w3", [H2D, H3D], f32, kind="ExternalInput")
    b3_d = nc.dram_tensor("b3", [H3D, 1], f32, kind="ExternalInput")
    w4_d = nc.dram_tensor("w4", [H3D, OUTD], f32, kind="ExternalInput")
    b4_d = nc.dram_tensor("b4", [OUTD, 1], f32, kind="ExternalInput")
    out_d = nc.dram_tensor("out", [p.npc, OUTD], f32, kind="ExternalOutput")

    h1_shard = nc.dram_tensor("h1_shard", [p.npc, H1D], edt)
    h1_full = nc.dram_tensor("h1_full", [p.n_nodes, H1D], edt,
                             addr_space="Shared")

    with tile.TileContext(nc) as tc:
        with (
            tc.tile_pool(name="const", bufs=1) as cpool,
            tc.tile_pool(name="gather", bufs=24) as gpool,
            tc.tile_pool(name="sel", bufs=3) as spool,
            tc.tile_pool(name="work", bufs=3) as wpool,
            tc.tile_pool(name="persist", bufs=1) as ppool,
            tc.tile_pool(name="psum", bufs=2, space="PSUM") as pspool,
            tc.tile_pool(name="psumfc", bufs=1, space="PSUM") as pfpool,
        ):
            # ---- constants / metadata to SBUF
            idx_s = cpool.tile([128, p.S // 16], i16)
            nc.sync.dma_start(idx_s[:], idx_d[:, :])
            sdiag_s = cpool.tile([128, p.n_win * p.win], edt)
            nc.sync.dma_start(sdiag_s[:], sdiag_d[:, :])
            w1_s = cpool.tile([F_IN, H1D], f32)
            nc.sync.dma_start(w1_s[:], w1_d[:, :])
            b1_s = cpool.tile([H1D, 1], f32)
            nc.sync.dma_start(b1_s[:], b1_d[:, :])
            w2_s = cpool.tile([H1D, H2D], f32)
            nc.sync.dma_start(w2_s[:], w2_d[:, :])
            b2_s = cpool.tile([H2D, 1], f32)
            nc.sync.dma_start(b2_s[:], b2_d[:, :])
            w3_s = cpool.tile([H2D, H3D], f32)
            nc.sync.dma_start(w3_s[:], w3_d[:, :])
            b3_s = cpool.tile([H3D, 1], f32)
            nc.sync.dma_start(b3_s[:], b3_d[:, :])
            w4_s = cpool.tile([H3D, OUTD], f32)
            nc.sync.dma_start(w4_s[:], w4_d[:, :])
            b4_s = cpool.tile([OUTD, 1], f32)
            nc.sync.dma_start(b4_s[:], b4_d[:, :])
            ident = cpool.tile([128, 128], f32)
            make_identity(nc, ident[:])

            h2T = ppool.tile([H2D, p.npc], f32)

            call_seq = [0]

            # ---------------- one GCN layer ----------------
            def gcn_layer(table_ap_fn, own_ap, w_s, b_s, out_feat, sink):
                issued = [0] * p.n_half
                bufs = {}

                def issue_next(hh):
                    ci = p.calls_h[hh][issued[hh]]
                    _, o, n = p.calls[ci]
                    gb = gpool.tile([128, (CALL // 128) * F_IN], edt, tag="gb")
                    out3 = gb[:].rearrange("q (t e) -> q t e", e=F_IN)
                    nc.gpsimd.dma_gather(
                        out_ap=out3[:, :n // 128, :],
                        in_ap=table_ap_fn(hh),
                        idxs_ap=idx_s[:, o // 16:(o + n) // 16],
                        num_idxs=n,
                        num_idxs_reg=n,
                        elem_size=F_IN,
                        single_packet=True,
                        queue_num=call_seq[0] % n_queues,
                    )
                    call_seq[0] += 1
                    bufs[ci] = gb
                    issued[hh] += 1

                def ensure(hh, local_needed, margin=10):
                    tgt = min(local_needed + margin, len(p.calls_h[hh]) - 1)
                    while issued[hh] <= tgt:
                        issue_next(hh)

                for ww in range(p.n_win):
                    for hh in range(p.n_half):
                        if p.win_need[ww][hh] >= 0:
                            ensure(hh, p.win_need[ww][hh])
                    q0, lst = p.win_ents[ww]
                    sbf = spool.tile([128, len(lst) * p.win], edt, tag="sbf")
                    nc.sync.dma_start(
                        sbf[:], smat_d[:, q0 * p.win:(q0 + len(lst)) * p.win])
                    wsz = p.win_sizes[ww]
                    xo = wpool.tile([128, F_IN], edt, tag="xo")
                    nc.sync.dma_start(
                        xo[:wsz, :], own_ap[ww * p.win: ww * p.win + wsz, :])
                    pag = pspool.tile([128, p.win], f32, tag="pag")
                    # self-loop term: x_own[window]^T @ diag(dinv^2)
                    nc.tensor.matmul(
                        pag[:], lhsT=xo[:wsz, :],
                        rhs=sdiag_s[:wsz, ww * p.win:(ww + 1) * p.win],
                        start=True, stop=(not lst))
                    for k, (hh, ci, tin) in enumerate(lst):
                        nc.tensor.matmul(
                            pag[:],
                            lhsT=bufs[ci][:, tin * F_IN:(tin + 1) * F_IN],
                            rhs=sbf[:, k * p.win:(k + 1) * p.win],
                            start=False, stop=(k == len(lst) - 1))
                    aggT = wpool.tile([128, p.win], f32, tag="aggT")
                    nc.vector.tensor_copy(aggT[:], pag[:])
                    ph = pspool.tile([out_feat, p.win], f32, tag="ph")
                    nc.tensor.matmul(ph[:], lhsT=w_s[:], rhs=aggT[:],
                                     start=True, stop=True)
                    sink(ww, wsz, ph, b_s)

            # ---- layer 1: x -> h1_shard (node-major, via PE transpose)
            def sink1(ww, wsz, ph, b_s):
                hT = wpool.tile([128, p.win], f32, tag="hT")
                nc.scalar.activation(hT[:], ph[:], AF.Tanh,
                                     bias=b_s[:, 0:1])
                pt = pspool.tile([128, 128], f32, tag="pt")
                nc.tensor.transpose(pt[:], hT[:], ident[:])
                hw_ = wpool.tile([128, 128], edt, tag="hw")
                nc.vector.tensor_copy(hw_[:], pt[:])
                nc.sync.dma_start(
                    h1_shard[ww * p.win: ww * p.win + wsz, :],
                    hw_[:wsz, :])

            gcn_layer(lambda hh: x_d[hh * p.half:
                                     min((hh + 1) * p.half, p.n_nodes), :],
                      xown_d, w1_s, b1_s, H1D, sink1)

            # ---- exchange h1 shards
            nc.gpsimd.collective_compute(
                "AllGather", mybir.AluOpType.bypass,
                replica_groups=[list(range(p.n_cores))],
                ins=[h1_shard[:, :]], outs=[h1_full[:, :]])

            # ---- layer 2: h1_full -> h2T (kept on-chip, feat-major)
            def sink2(ww, wsz, ph, b_s):
                nc.scalar.activation(
                    h2T[:, ww * p.win: ww * p.win + wsz],
                    ph[:, :wsz], AF.Tanh, bias=b_s[:, 0:1])

            gcn_layer(lambda hh: h1_full[hh * p.half:
                                         min((hh + 1) * p.half, p.n_nodes), :],
                      h1_shard, w2_s, b2_s, H2D, sink2)

            # ---- fc layers on the dst shard
            for c0 in range(0, p.npc, NCHUNK):
                cs = min(NCHUNK, p.npc - c0)
                p3 = pfpool.tile([H3D, NCHUNK], f32, tag="p3")
                nc.tensor.matmul(p3[:, :cs], lhsT=w3_s[:],
                                 rhs=h2T[:, c0:c0 + cs],
                                 start=True, stop=True)
                h3 = wpool.tile([H3D, NCHUNK], f32, tag="h3")
                nc.scalar.activation(h3[:, :cs], p3[:, :cs], AF.Tanh,
                                     bias=b3_s[:, 0:1])
                p4 = pfpool.tile([OUTD, NCHUNK], f32, tag="p4")
                nc.tensor.matmul(p4[:, :cs], lhsT=w4_s[:], rhs=h3[:, :cs],
                                 start=True, stop=True)
                ob = wpool.tile([OUTD, NCHUNK], f32, tag="ob")
                nc.vector.tensor_scalar(
                    out=ob[:, :cs], in0=p4[:, :cs],
                    scalar1=b4_s[0:1, 0:1], scalar2=None, op0=OP.add)
                nc.sync.dma_start(out_d[c0:c0 + cs, :], ob[0:1, :cs])

    nc.compile()
    return nc


def make_in_maps(p, inputs, edge_dt="float16"):
    np_edt = dict(float32=np.float32, float16=np.float16)[edge_dt]
    x = np.ascontiguousarray(np.asarray(inputs["x"]).astype(np_edt))
    maps = []
    for c in range(p.n_cores):
        maps.append({
            "x": x,
            "midx": p.idx_arr[c],
            "smat": p.smat[c],
            "sdiag": p.sdiag[c],
            "xown": x[c * p.npc:(c + 1) * p.npc],
            "w1": np.asarray(inputs["W1"], dtype=np.float32),
            "b1": np.asarray(inputs["b1"], dtype=np.float32).reshape(-1, 1),
            "w2": np.asarray(inputs["W2"], dtype=np.float32),
            "b2": np.asarray(inputs["b2"], dtype=np.float32).reshape(-1, 1),
            "w3": np.asarray(inputs["W3"], dtype=np.float32),
            "b3": np.asarray(inputs["b3"], dtype=np.float32).reshape(-1, 1),
            "w4": np.asarray(inputs["W4"], dtype=np.float32),
            "b4": np.asarray(inputs["b4"], dtype=np.float32).reshape(-1, 1),
        })
    return maps


_CACHE = {}


def kernel(_trace=False, **inputs):
    from concourse.bass_utils import run_bass_kernel_spmd

    edge_index = np.asarray(inputs["edge_index"])
    p = make_plan(edge_index)
    key = (p.S, tuple(int(c[2]) for c in p.calls))
    if key not in _CACHE:
        _CACHE[key] = build_program(p)
    nc = _CACHE[key]
    res = run_bass_kernel_spmd(nc, make_in_maps(p, inputs),
                               core_ids=list(range(p.n_cores)),
                               trace=_trace)
    out = np.concatenate([res.results[c]["out"] for c in range(p.n_cores)],
                         axis=0)
    if _trace:
        return out, res
    return out


# revision 16
# speedup vs baseline: 1.0722x; 1.0722x over previous
"""BrainGCN Trainium2 kernel (8 NeuronCores, Bass/Tile).

Model (PyG-style GCNConv x2 + 2 FC layers):
    h = tanh(gcn(x,  W1, b1)); h = tanh(gcn(h, W2, b2))
    h = tanh(h @ W3 + b3);      out = h @ W4 + b4

gcn(x, W, b) = (agg + x * dinv^2) @ W + b  with
    agg[d] = sum_{e:(s,d)} dinv[s]*dinv[d] * x[s]        (by linearity we
aggregate raw feature rows first, then apply W once per node).

Distribution: dst-nodes are split into 8 contiguous blocks (one per core).
Each core aggregates its own dst block; self-loops are folded in via a
host-built diagonal S block.  The only cross-core exchange is an AllGather
of the h1 shards between the two GCN layers.

Edge slots are laid out in two half-major regions (src < / >= HALF so the
int16 gather indices stay in range), window-major inside each region.  The
regions are gathered by uniform 1024-slot dma_gather calls (single_packet
concat chains cap out at 64 descriptors = 16KB per SDMA lane) that rotate
over the 4 SWDGE queues; windows then accumulate tiles from both regions
into one PSUM bank, so no extra pass is needed.  For every 128-slot tile
the PE accumulates aggT[feat, dst] += E_tile^T @ S into PSUM where S is a
host-built coef-scattered block streamed from HBM.
"""

import numpy as np

# ---------------------------------------------------------------- constants
N_NODES = 50000
N_CORES = 8
F_IN, H1D, H2D, H3D, OUTD = 128, 128, 64, 64, 1
WIN = 128          # dst window width (psum free dim of the scatter matmul)
HALF = 25000       # gather-table region size (int16 index range)
CALL = 1024        # gather slots per dma_gather call (64 descs per lane)
NCHUNK = 512       # fc-layer column chunk


def _cdiv(a, b):
    return -(-a // b)


def _rup(a, b):
    return _cdiv(a, b) * b


# ------------------------------------------------------------------ planning
class Plan:
    pass


def make_plan(edge_index, n_nodes=N_NODES, n_cores=N_CORES, half=HALF,
              win=WIN, call_sz=CALL):
    """Host-side graph preprocessing -> static schedule + per-core arrays."""
    src = np.asarray(edge_index[0]).astype(np.int64)
    dst = np.asarray(edge_index[1]).astype(np.int64)

    npc = n_nodes // n_cores
    assert npc * n_cores == n_nodes
    n_win = _cdiv(npc, win)
    n_half = _cdiv(n_nodes, half)
    assert half <= 32767

    deg = np.bincount(dst, minlength=n_nodes).astype(np.float64) + 1.0
    dinv = 1.0 / np.sqrt(deg)

    s_all, d_all = src, dst
    coef = (dinv[s_all] * dinv[d_all]).astype(np.float32)

    core = d_all // npc
    h = s_all // half

    # ---- balanced window packing: permute each core's dst nodes across its
    # windows so per-(window, half) edge counts are nearly equal across cores
    # (caps are max-over-cores, so imbalance directly pads the slot layout).
    degh = np.zeros((n_nodes, n_half), np.int64)
    np.add.at(degh, (d_all, h), 1)
    win_sizes = [min(win, npc - ww * win) for ww in range(n_win)]
    cap_arr = np.array(win_sizes)
    wof = np.empty(n_nodes, np.int64)
    dlocof = np.empty(n_nodes, np.int64)
    for c in range(n_cores):
        ids = np.arange(c * npc, (c + 1) * npc)
        order = ids[np.argsort(-degh[ids].sum(1), kind="stable")]
        loads = np.zeros((n_win, n_half))
        cnt = np.zeros(n_win, np.int64)
        for d in order:
            sc = np.max(loads + degh[d][None, :], axis=1)
            sc[cnt >= cap_arr] = np.inf
            wsel = int(np.argmin(sc))
            wof[d] = wsel
            dlocof[d] = cnt[wsel]
            cnt[wsel] += 1
            loads[wsel] += degh[d]
    # perm[pos] = original node id at packed position pos
    perm = np.empty(n_nodes, np.int64)
    perm[(np.arange(n_nodes) // npc) * npc + wof * win + dlocof] = \
        np.arange(n_nodes)

    w = wof[d_all]

    # per (core, w, h) counts -> static caps shared by all cores (SPMD)
    gid = (core * n_win + w) * n_half + h
    counts = np.bincount(gid, minlength=n_cores * n_win * n_half)
    caps = counts.reshape(n_cores, n_win, n_half).max(axis=0)  # [n_win,n_half]

    # ---- half-major slot regions, window-major inside
    off = np.zeros((n_win, n_half), dtype=np.int64)
    regions = []                  # (base, real_end, padded_end) per half
    pos = 0
    for hh in range(n_half):
        base = pos
        for ww in range(n_win):
            off[ww, hh] = pos
            pos += int(caps[ww, hh])
        real_end = pos
        pos = _rup(pos, 128)
        regions.append((base, real_end, pos))
    S = pos
    T = S // 128

    # ---- uniform gather calls per region
    calls = []                    # (h, slot_off, n_slots)
    calls_h = [[] for _ in range(n_half)]
    for hh, (base, _re, pend) in enumerate(regions):
        o = base
        while o < pend:
            n = min(call_sz, pend - o)
            calls_h[hh].append(len(calls))
            calls.append((hh, o, n))
            o += n
    call_of_tile = {}             # global tile -> (call_global, tile_in_call)
    for ci, (hh, o, n) in enumerate(calls):
        for t in range(n // 128):
            call_of_tile[o // 128 + t] = (ci, t)
    local_of_call = {}            # global call -> region-local index
    for hh in range(n_half):
        for li, ci in enumerate(calls_h[hh]):
            local_of_call[ci] = li

    # ---- per-window tile/ent lists in processing order
    ents = []                     # (ww, hh, ci, tin, t_global)
    win_ents = []                 # per window: (q0, [(hh, ci, tin)...])
    win_need = []                 # per window: max region-local call idx per h
    for ww in range(n_win):
        lst = []
        need = [-1] * n_half
        for hh in range(n_half):
            o, cp = int(off[ww, hh]), int(caps[ww, hh])
            if cp == 0:
                continue
            for t in range(o // 128, (o + cp - 1) // 128 + 1):
                ci, tin = call_of_tile[t]
                lst.append((hh, ci, tin))
                need[hh] = max(need[hh], local_of_call[ci])
                ents.append((ww, hh, ci, tin, t))
        win_ents.append((len(ents) - len(lst), lst))
        win_need.append(need)
    NE = len(ents)

    # ---- per-core arrays: order edges by (core, h, w), rank within group
    order_key = (core * n_half + h) * n_win + w
    order = np.argsort(order_key, kind="stable")
    s_o, d_o, c_o = s_all[order], d_all[order], coef[order]
    core_o, w_o, h_o = core[order], w[order], h[order]

    comb = (core_o * n_half + h_o) * n_win + w_o
    pos_a = np.arange(comb.size, dtype=np.int64)
    is_start = np.ones(comb.size, dtype=bool)
    if comb.size > 1:
        is_start[1:] = comb[1:] != comb[:-1]
    rank = pos_a - np.maximum.accumulate(np.where(is_start, pos_a, 0))
    slot = off[w_o, h_o] + rank

    idx16 = np.zeros((n_cores, S), dtype=np.int16)
    idx16b = np.zeros((n_cores, S), dtype=np.int16)
    dloc = np.zeros((n_cores, S), dtype=np.int64)
    cof = np.zeros((n_cores, S), dtype=np.float32)
    ci_core = core_o.astype(np.int64)
    idx16[ci_core, slot] = (s_o - h_o * half).astype(np.int16)
    # layer-2 table (h1_full) rows live at packed positions
    pos2 = (s_o // npc) * npc + wof[s_o] * win + dlocof[s_o]
    assert np.all(pos2 // half == h_o)
    idx16b[ci_core, slot] = (pos2 - h_o * half).astype(np.int16)
    dloc[ci_core, slot] = dlocof[d_o]
    cof[ci_core, slot] = c_o
    for (_b, real_end, pend) in regions:
        idx16[:, real_end:pend] = -1       # trailing pads: ucode trims them
        idx16b[:, real_end:pend] = -1

    p = Plan()
    p.n_nodes, p.n_cores, p.npc = n_nodes, n_cores, npc
    p.win, p.n_win, p.half, p.n_half = win, n_win, half, n_half
    p.win_sizes = win_sizes
    p.calls, p.calls_h = calls, calls_h
    p.win_ents, p.win_need = win_ents, win_need
    p.S, p.T, p.NE = S, T, NE
    p.perm = perm
    # idx layout for the device: [128, 2*S/16] (16-row wrap, replicated x8;
    # layer-1 indices first, then layer-2 indices)
    def _wrap(a):
        return np.tile(a.reshape(n_cores, S // 16, 16).transpose(0, 2, 1),
                       (1, 8, 1))
    p.idx_arr = np.ascontiguousarray(
        np.concatenate([_wrap(idx16), _wrap(idx16b)], axis=2))

    # window owner of each static slot (pad slots: -1)
    w_of_slot = np.full(S, -1, dtype=np.int64)
    for ww in range(n_win):
        for hh in range(n_half):
            o = int(off[ww, hh])
            w_of_slot[o:o + int(caps[ww, hh])] = ww

    # host-built S tiles in ents order:
    # smat[c, p, q*win + dloc] = coef if slot t*128+p belongs to window ww
    smat = np.zeros((n_cores, 128, NE * win), dtype=np.float16)
    cidx = np.arange(n_cores)[:, None]
    for q, (ww, hh, ci, tin, t) in enumerate(ents):
        sl = np.arange(t * 128, t * 128 + 128)
        m = w_of_slot[sl] == ww
        rows = np.where(m)[0]
        if rows.size == 0:
            continue
        smat[cidx, rows[None, :], q * win + dloc[:, sl[m]]] = cof[:, sl[m]]
    p.smat = np.ascontiguousarray(smat)

    # self-loop diagonal: sdiag[c, p, w*win + j] = (p==j)*dinv^2[global node]
    d2 = (dinv * dinv).astype(np.float32)
    sdiag = np.zeros((n_cores, 128, n_win * win), dtype=np.float16)
    for c in range(n_cores):
        for ww in range(n_win):
            wsz = min(win, npc - ww * win)
            g0 = c * npc + ww * win
            sdiag[c, np.arange(wsz), ww * win + np.arange(wsz)] = \
                d2[perm[g0:g0 + wsz]]
    p.sdiag = sdiag

    return p


# ------------------------------------------------------------------- program
def build_program(p, debug=False, n_queues=4, scratch=32768,
                  edge_dt="float16"):
    import concourse.bacc as bacc
    import concourse.mybir as mybir
    import concourse.tile as tile
    from concourse.masks import make_identity

    f32 = mybir.dt.float32
    edt = getattr(mybir.dt, edge_dt)
    i16 = mybir.dt.int16
    AF = mybir.ActivationFunctionType
    OP = mybir.AluOpType

    nc = bacc.Bacc("TRN2", target_bir_lowering=False, debug=debug,
                   num_devices=p.n_cores, num_swdge_queues=n_queues,
                   dynamic_dma_scratch_size=scratch)

    x_d = nc.dram_tensor("x", [p.n_nodes, F_IN], edt, kind="ExternalInput")
    idx_d = nc.dram_tensor("midx", [128, 2 * (p.S // 16)], i16,
                           kind="ExternalInput")
    smat_d = nc.dram_tensor("smat", [128, p.NE * p.win], edt,
                            kind="ExternalInput")
    sdiag_d = nc.dram_tensor("sdiag", [128, p.n_win * p.win], edt,
                             kind="ExternalInput")
    xown_d = nc.dram_tensor("xown", [p.npc, F_IN], edt, kind="ExternalInput")
    w1_d = nc.dram_tensor("w1", [F_IN, H1D], f32, kind="ExternalInput")
    b1_d = nc.dram_tensor("b1", [H1D, 1], f32, kind="ExternalInput")
    w2_d = nc.dram_tensor("w2", [H1D, H2D], f32, kind="ExternalInput")
    b2_d = nc.dram_tensor("b2", [H2D, 1], f32, kind="ExternalInput")
    w3_d = nc.dram_tensor("w3", [H2D, H3D], f32, kind="ExternalInput")
    b3_d = nc.dram_tensor("b3", [H3D, 1], f32, kind="ExternalInput")
    w4_d = nc.dram_tensor("w4", [H3D, OUTD], f32, kind="ExternalInput")
    b4_d = nc.dram_tensor("b4", [OUTD, 1], f32, kind="ExternalInput")
    out_d = nc.dram_tensor("out", [p.npc, OUTD], f32, kind="ExternalOutput")

    h1_shard = nc.dram_tensor("h1_shard", [p.npc, H1D], edt)
    h1_full = nc.dram_tensor("h1_full", [p.n_nodes, H1D], edt,
                             addr_space="Shared")

    with tile.TileContext(nc) as tc:
        with (
            tc.tile_pool(name="const", bufs=1) as cpool,
            tc.tile_pool(name="gather", bufs=16) as gpool,
            tc.tile_pool(name="sel", bufs=3) as spool,
            tc.tile_pool(name="work", bufs=3) as wpool,
            tc.tile_pool(name="persist", bufs=1) as ppool,
            tc.tile_pool(name="psum", bufs=2, space="PSUM") as pspool,
            tc.tile_pool(name="psumfc", bufs=1, space="PSUM") as pfpool,
        ):
            # ---- constants / metadata to SBUF
            idx_s = cpool.tile([128, 2 * (p.S // 16)], i16)
            nc.sync.dma_start(idx_s[:], idx_d[:, :])
            sdiag_s = cpool.tile([128, p.n_win * p.win], edt)
            nc.sync.dma_start(sdiag_s[:], sdiag_d[:, :])
            w1_s = cpool.tile([F_IN, H1D], f32)
            nc.sync.dma_start(w1_s[:], w1_d[:, :])
            b1_s = cpool.tile([H1D, 1], f32)
            nc.sync.dma_start(b1_s[:], b1_d[:, :])
            w2_s = cpool.tile([H1D, H2D], f32)
            nc.sync.dma_start(w2_s[:], w2_d[:, :])
            b2_s = cpool.tile([H2D, 1], f32)
            nc.sync.dma_start(b2_s[:], b2_d[:, :])
            w3_s = cpool.tile([H2D, H3D], f32)
            nc.sync.dma_start(w3_s[:], w3_d[:, :])
            b3_s = cpool.tile([H3D, 1], f32)
            nc.sync.dma_start(b3_s[:], b3_d[:, :])
            w4_s = cpool.tile([H3D, OUTD], f32)
            nc.sync.dma_start(w4_s[:], w4_d[:, :])
            b4_s = cpool.tile([OUTD, 1], f32)
            nc.sync.dma_start(b4_s[:], b4_d[:, :])
            ident = cpool.tile([128, 128], f32)
            make_identity(nc, ident[:])

            h2T = ppool.tile([H2D, p.npc], f32)

            call_seq = [0]

            # ---------------- one GCN layer ----------------
            def gcn_layer(table_ap_fn, own_ap, w_s, b_s, out_feat, sink,
                          idx_base=0):
                issued = [0] * p.n_half
                bufs = {}

                def issue_next(hh):
                    ci = p.calls_h[hh][issued[hh]]
                    _, o, n = p.calls[ci]
                    gb = gpool.tile([128, (CALL // 128) * F_IN], edt, tag="gb")
                    out3 = gb[:].rearrange("q (t e) -> q t e", e=F_IN)
                    nc.gpsimd.dma_gather(
                        out_ap=out3[:, :n // 128, :],
                        in_ap=table_ap_fn(hh),
                        idxs_ap=idx_s[:, idx_base + o // 16:
                                      idx_base + (o + n) // 16],
                        num_idxs=n,
                        num_idxs_reg=n,
                        elem_size=F_IN,
                        single_packet=True,
                        queue_num=call_seq[0] % n_queues,
                    )
                    call_seq[0] += 1
                    bufs[ci] = gb
                    issued[hh] += 1

                def ensure(hh, local_needed, margin=5):
                    tgt = min(local_needed + margin, len(p.calls_h[hh]) - 1)
                    while issued[hh] <= tgt:
                        issue_next(hh)

                for ww in range(p.n_win):
                    for hh in range(p.n_half):
                        if p.win_need[ww][hh] >= 0:
                            ensure(hh, p.win_need[ww][hh])
                    q0, lst = p.win_ents[ww]
                    sbf = spool.tile([128, len(lst) * p.win], edt, tag="sbf")
                    nc.sync.dma_start(
                        sbf[:], smat_d[:, q0 * p.win:(q0 + len(lst)) * p.win])
                    wsz = p.win_sizes[ww]
                    xo = wpool.tile([128, F_IN], edt, tag="xo")
                    nc.sync.dma_start(
                        xo[:wsz, :], own_ap[ww * p.win: ww * p.win + wsz, :])
                    pag = pspool.tile([128, p.win], f32, tag="pag")
                    # self-loop term: x_own[window]^T @ diag(dinv^2)
                    nc.tensor.matmul(
                        pag[:], lhsT=xo[:wsz, :],
                        rhs=sdiag_s[:wsz, ww * p.win:(ww + 1) * p.win],
                        start=True, stop=(not lst))
                    for k, (hh, ci, tin) in enumerate(lst):
                        nc.tensor.matmul(
                            pag[:],
                            lhsT=bufs[ci][:, tin * F_IN:(tin + 1) * F_IN],
                            rhs=sbf[:, k * p.win:(k + 1) * p.win],
                            start=False, stop=(k == len(lst) - 1))
                    aggT = wpool.tile([128, p.win], f32, tag="aggT")
                    nc.vector.tensor_copy(aggT[:], pag[:])
                    ph = pspool.tile([out_feat, p.win], f32, tag="ph")
                    nc.tensor.matmul(ph[:], lhsT=w_s[:], rhs=aggT[:],
                                     start=True, stop=True)
                    sink(ww, wsz, ph, b_s)

            # ---- layer 1: x -> h1_shard (node-major, via PE transpose)
            def sink1(ww, wsz, ph, b_s):
                hT = wpool.tile([128, p.win], f32, tag="hT")
                nc.scalar.activation(hT[:], ph[:], AF.Tanh,
                                     bias=b_s[:, 0:1])
                pt = pspool.tile([128, 128], f32, tag="pt")
                nc.tensor.transpose(pt[:], hT[:], ident[:])
                hw_ = wpool.tile([128, 128], edt, tag="hw")
                nc.vector.tensor_copy(hw_[:], pt[:])
                nc.sync.dma_start(
                    h1_shard[ww * p.win: ww * p.win + wsz, :],
                    hw_[:wsz, :])

            gcn_layer(lambda hh: x_d[hh * p.half:
                                     min((hh + 1) * p.half, p.n_nodes), :],
                      xown_d, w1_s, b1_s, H1D, sink1)

            # ---- exchange h1 shards
            nc.gpsimd.collective_compute(
                "AllGather", mybir.AluOpType.bypass,
                replica_groups=[list(range(p.n_cores))],
                ins=[h1_shard[:, :]], outs=[h1_full[:, :]])

            # ---- layer 2: h1_full -> h2T (kept on-chip, feat-major)
            def sink2(ww, wsz, ph, b_s):
                nc.scalar.activation(
                    h2T[:, ww * p.win: ww * p.win + wsz],
                    ph[:, :wsz], AF.Tanh, bias=b_s[:, 0:1])

            gcn_layer(lambda hh: h1_full[hh * p.half:
                                         min((hh + 1) * p.half, p.n_nodes), :],
                      h1_shard, w2_s, b2_s, H2D, sink2,
                      idx_base=p.S // 16)

            # ---- fc layers on the dst shard
            for c0 in range(0, p.npc, NCHUNK):
                cs = min(NCHUNK, p.npc - c0)
                p3 = pfpool.tile([H3D, NCHUNK], f32, tag="p3")
                nc.tensor.matmul(p3[:, :cs], lhsT=w3_s[:],
                                 rhs=h2T[:, c0:c0 + cs],
                                 start=True, stop=True)
                h3 = wpool.tile([H3D, NCHUNK], f32, tag="h3")
                nc.scalar.activation(h3[:, :cs], p3[:, :cs], AF.Tanh,
                                     bias=b3_s[:, 0:1])
                p4 = pfpool.tile([OUTD, NCHUNK], f32, tag="p4")
                nc.tensor.matmul(p4[:, :cs], lhsT=w4_s[:], rhs=h3[:, :cs],
                                 start=True, stop=True)
                ob = wpool.tile([OUTD, NCHUNK], f32, tag="ob")
                nc.vector.tensor_scalar(
                    out=ob[:, :cs], in0=p4[:, :cs],
                    scalar1=b4_s[0:1, 0:1], scalar2=None, op0=OP.add)
                nc.sync.dma_start(out_d[c0:c0 + cs, :], ob[0:1, :cs])

    nc.compile()
    return nc


def make_in_maps(p, inputs, edge_dt="float16"):
    np_edt = dict(float32=np.float32, float16=np.float16)[edge_dt]
    x = np.ascontiguousarray(np.asarray(inputs["x"]).astype(np_edt))
    maps = []
    for c in range(p.n_cores):
        maps.append({
            "x": x,
            "midx": p.idx_arr[c],
            "smat": p.smat[c],
            "sdiag": p.sdiag[c],
            "xown": np.ascontiguousarray(
                x[p.perm[c * p.npc:(c + 1) * p.npc]]),
            "w1": np.asarray(inputs["W1"], dtype=np.float32),
            "b1": np.asarray(inputs["b1"], dtype=np.float32).reshape(-1, 1),
            "w2": np.asarray(inputs["W2"], dtype=np.float32),
            "b2": np.asarray(inputs["b2"], dtype=np.float32).reshape(-1, 1),
            "w3": np.asarray(inputs["W3"], dtype=np.float32),
            "b3": np.asarray(inputs["b3"], dtype=np.float32).reshape(-1, 1),
            "w4": np.asarray(inputs["W4"], dtype=np.float32),
            "b4": np.asarray(inputs["b4"], dtype=np.float32).reshape(-1, 1),
        })
    return maps


_CACHE = {}


def kernel(_trace=False, **inputs):
    from concourse.bass_utils import run_bass_kernel_spmd

    edge_index = np.asarray(inputs["edge_index"])
    p = make_plan(edge_index)
    key = (p.S, tuple(int(c[2]) for c in p.calls))
    if key not in _CACHE:
        _CACHE[key] = build_program(p)
    nc = _CACHE[key]
    res = run_bass_kernel_spmd(nc, make_in_maps(p, inputs),
                               core_ids=list(range(p.n_cores)),
                               trace=_trace)
    res_cat = np.concatenate(
        [res.results[c]["out"] for c in range(p.n_cores)], axis=0)
    out = np.empty_like(res_cat)
    out[p.perm] = res_cat
    if _trace:
        return out, res
    return out


# revision 19
# speedup vs baseline: 1.0877x; 1.0144x over previous
"""BrainGCN Trainium2 kernel (8 NeuronCores, Bass/Tile).

Model (PyG-style GCNConv x2 + 2 FC layers):
    h = tanh(gcn(x,  W1, b1)); h = tanh(gcn(h, W2, b2))
    h = tanh(h @ W3 + b3);      out = h @ W4 + b4

gcn(x, W, b) = (agg + x * dinv^2) @ W + b  with
    agg[d] = sum_{e:(s,d)} dinv[s]*dinv[d] * x[s]        (by linearity we
aggregate raw feature rows first, then apply W once per node).

Distribution: dst-nodes are split into 8 contiguous blocks (one per core).
Each core aggregates its own dst block; self-loops are folded in via a
host-built diagonal S block.  The only cross-core exchange is an AllGather
of the h1 shards between the two GCN layers.

Edge slots are laid out in two half-major regions (src < / >= HALF so the
int16 gather indices stay in range), window-major inside each region.  The
regions are gathered by uniform 1024-slot dma_gather calls (single_packet
concat chains cap out at 64 descriptors = 16KB per SDMA lane) that rotate
over the 4 SWDGE queues; windows then accumulate tiles from both regions
into one PSUM bank, so no extra pass is needed.  For every 128-slot tile
the PE accumulates aggT[feat, dst] += E_tile^T @ S into PSUM where S is a
host-built coef-scattered block streamed from HBM.
"""

import numpy as np

# ---------------------------------------------------------------- constants
N_NODES = 50000
N_CORES = 8
F_IN, H1D, H2D, H3D, OUTD = 128, 128, 64, 64, 1
WIN = 128          # dst window width (psum free dim of the scatter matmul)
HALF = 25000       # gather-table region size (int16 index range)
CALL = 1024        # gather slots per dma_gather call (64 descs per lane)
NCHUNK = 512       # fc-layer column chunk


def _cdiv(a, b):
    return -(-a // b)


def _rup(a, b):
    return _cdiv(a, b) * b


# ------------------------------------------------------------------ planning
class Plan:
    pass


def make_plan(edge_index, n_nodes=N_NODES, n_cores=N_CORES, half=HALF,
              win=WIN, call_sz=CALL):
    """Host-side graph preprocessing -> static schedule + per-core arrays."""
    src = np.asarray(edge_index[0]).astype(np.int64)
    dst = np.asarray(edge_index[1]).astype(np.int64)

    npc = n_nodes // n_cores
    assert npc * n_cores == n_nodes
    n_win = _cdiv(npc, win)
    n_half = _cdiv(n_nodes, half)
    assert half <= 32767

    deg = np.bincount(dst, minlength=n_nodes).astype(np.float64) + 1.0
    dinv = 1.0 / np.sqrt(deg)

    s_all, d_all = src, dst
    coef = (dinv[s_all] * dinv[d_all]).astype(np.float32)

    core = d_all // npc
    h = s_all // half

    # ---- balanced window packing: permute each core's dst nodes across its
    # windows so per-(window, half) edge counts are nearly equal across cores
    # (caps are max-over-cores, so imbalance directly pads the slot layout).
    degh = np.zeros((n_nodes, n_half), np.int64)
    np.add.at(degh, (d_all, h), 1)
    win_sizes = [min(win, npc - ww * win) for ww in range(n_win)]
    cap_arr = np.array(win_sizes)
    wof = np.empty(n_nodes, np.int64)
    dlocof = np.empty(n_nodes, np.int64)
    for c in range(n_cores):
        ids = np.arange(c * npc, (c + 1) * npc)
        order = ids[np.argsort(-degh[ids].sum(1), kind="stable")]
        loads = np.zeros((n_win, n_half))
        cnt = np.zeros(n_win, np.int64)
        for d in order:
            sc = np.max(loads + degh[d][None, :], axis=1)
            sc[cnt >= cap_arr] = np.inf
            wsel = int(np.argmin(sc))
            wof[d] = wsel
            dlocof[d] = cnt[wsel]
            cnt[wsel] += 1
            loads[wsel] += degh[d]
    # perm[pos] = original node id at packed position pos
    perm = np.empty(n_nodes, np.int64)
    perm[(np.arange(n_nodes) // npc) * npc + wof * win + dlocof] = \
        np.arange(n_nodes)

    w = wof[d_all]

    # per (core, w, h) counts -> static caps shared by all cores (SPMD)
    gid = (core * n_win + w) * n_half + h
    counts = np.bincount(gid, minlength=n_cores * n_win * n_half)
    caps = counts.reshape(n_cores, n_win, n_half).max(axis=0)  # [n_win,n_half]

    # ---- half-major slot regions, window-major inside
    off = np.zeros((n_win, n_half), dtype=np.int64)
    regions = []                  # (base, real_end, padded_end) per half
    pos = 0
    for hh in range(n_half):
        base = pos
        for ww in range(n_win):
            off[ww, hh] = pos
            pos += int(caps[ww, hh])
        real_end = pos
        pos = _rup(pos, 128)
        regions.append((base, real_end, pos))
    S = pos
    T = S // 128

    # ---- uniform gather calls per region
    calls = []                    # (h, slot_off, n_slots)
    calls_h = [[] for _ in range(n_half)]
    for hh, (base, _re, pend) in enumerate(regions):
        o = base
        while o < pend:
            n = min(call_sz, pend - o)
            calls_h[hh].append(len(calls))
            calls.append((hh, o, n))
            o += n
    call_of_tile = {}             # global tile -> (call_global, tile_in_call)
    for ci, (hh, o, n) in enumerate(calls):
        for t in range(n // 128):
            call_of_tile[o // 128 + t] = (ci, t)
    local_of_call = {}            # global call -> region-local index
    for hh in range(n_half):
        for li, ci in enumerate(calls_h[hh]):
            local_of_call[ci] = li

    # ---- per-window tile/ent lists in processing order
    ents = []                     # (ww, hh, ci, tin, t_global)
    win_ents = []                 # per window: (q0, [(hh, ci, tin)...])
    win_need = []                 # per window: max region-local call idx per h
    for ww in range(n_win):
        lst = []
        need = [-1] * n_half
        for hh in range(n_half):
            o, cp = int(off[ww, hh]), int(caps[ww, hh])
            if cp == 0:
                continue
            for t in range(o // 128, (o + cp - 1) // 128 + 1):
                ci, tin = call_of_tile[t]
                lst.append((hh, ci, tin))
                need[hh] = max(need[hh], local_of_call[ci])
                ents.append((ww, hh, ci, tin, t))
        win_ents.append((len(ents) - len(lst), lst))
        win_need.append(need)
    NE = len(ents)

    # ---- per-core arrays: order edges by (core, h, w), rank within group
    order_key = (core * n_half + h) * n_win + w
    order = np.argsort(order_key, kind="stable")
    s_o, d_o, c_o = s_all[order], d_all[order], coef[order]
    core_o, w_o, h_o = core[order], w[order], h[order]

    comb = (core_o * n_half + h_o) * n_win + w_o
    pos_a = np.arange(comb.size, dtype=np.int64)
    is_start = np.ones(comb.size, dtype=bool)
    if comb.size > 1:
        is_start[1:] = comb[1:] != comb[:-1]
    rank = pos_a - np.maximum.accumulate(np.where(is_start, pos_a, 0))
    slot = off[w_o, h_o] + rank

    idx16 = np.zeros((n_cores, S), dtype=np.int16)
    idx16b = np.zeros((n_cores, S), dtype=np.int16)
    dloc = np.zeros((n_cores, S), dtype=np.int64)
    cof = np.zeros((n_cores, S), dtype=np.float32)
    ci_core = core_o.astype(np.int64)
    idx16[ci_core, slot] = (s_o - h_o * half).astype(np.int16)
    # layer-2 table (h1_full) rows live at packed positions
    pos2 = (s_o // npc) * npc + wof[s_o] * win + dlocof[s_o]
    assert np.all(pos2 // half == h_o)
    idx16b[ci_core, slot] = (pos2 - h_o * half).astype(np.int16)
    dloc[ci_core, slot] = dlocof[d_o]
    cof[ci_core, slot] = c_o
    for (_b, real_end, pend) in regions:
        idx16[:, real_end:pend] = -1       # trailing pads: ucode trims them
        idx16b[:, real_end:pend] = -1

    p = Plan()
    p.n_nodes, p.n_cores, p.npc = n_nodes, n_cores, npc
    p.win, p.n_win, p.half, p.n_half = win, n_win, half, n_half
    p.win_sizes = win_sizes
    p.calls, p.calls_h = calls, calls_h
    p.win_ents, p.win_need = win_ents, win_need
    p.S, p.T, p.NE = S, T, NE
    p.perm = perm
    # idx layout for the device: [128, 2*S/16] (16-row wrap, replicated x8;
    # layer-1 indices first, then layer-2 indices)
    def _wrap(a):
        return np.tile(a.reshape(n_cores, S // 16, 16).transpose(0, 2, 1),
                       (1, 8, 1))
    p.idx_arr = np.ascontiguousarray(
        np.concatenate([_wrap(idx16), _wrap(idx16b)], axis=2))

    # window owner of each static slot (pad slots: -1)
    w_of_slot = np.full(S, -1, dtype=np.int64)
    for ww in range(n_win):
        for hh in range(n_half):
            o = int(off[ww, hh])
            w_of_slot[o:o + int(caps[ww, hh])] = ww

    # host-built S tiles in ents order:
    # smat[c, p, q*win + dloc] = coef if slot t*128+p belongs to window ww
    smat = np.zeros((n_cores, 128, NE * win), dtype=np.float16)
    cidx = np.arange(n_cores)[:, None]
    for q, (ww, hh, ci, tin, t) in enumerate(ents):
        sl = np.arange(t * 128, t * 128 + 128)
        m = w_of_slot[sl] == ww
        rows = np.where(m)[0]
        if rows.size == 0:
            continue
        smat[cidx, rows[None, :], q * win + dloc[:, sl[m]]] = cof[:, sl[m]]
    p.smat = np.ascontiguousarray(smat)

    # self-loop diagonal: sdiag[c, p, w*win + j] = (p==j)*dinv^2[global node]
    d2 = (dinv * dinv).astype(np.float32)
    sdiag = np.zeros((n_cores, 128, n_win * win), dtype=np.float16)
    for c in range(n_cores):
        for ww in range(n_win):
            wsz = min(win, npc - ww * win)
            g0 = c * npc + ww * win
            sdiag[c, np.arange(wsz), ww * win + np.arange(wsz)] = \
                d2[perm[g0:g0 + wsz]]
    p.sdiag = sdiag

    return p


# ------------------------------------------------------------------- program
def build_program(p, debug=False, n_queues=4, scratch=32768,
                  edge_dt="float16"):
    import concourse.bacc as bacc
    import concourse.mybir as mybir
    import concourse.tile as tile
    from concourse.masks import make_identity

    f32 = mybir.dt.float32
    edt = getattr(mybir.dt, edge_dt)
    i16 = mybir.dt.int16
    AF = mybir.ActivationFunctionType
    OP = mybir.AluOpType

    nc = bacc.Bacc("TRN2", target_bir_lowering=False, debug=debug,
                   num_devices=p.n_cores, num_swdge_queues=n_queues,
                   dynamic_dma_scratch_size=scratch)

    x_d = nc.dram_tensor("x", [p.n_nodes, F_IN], edt, kind="ExternalInput")
    idx_d = nc.dram_tensor("midx", [128, 2 * (p.S // 16)], i16,
                           kind="ExternalInput")
    smat_d = nc.dram_tensor("smat", [128, p.NE * p.win], edt,
                            kind="ExternalInput")
    sdiag_d = nc.dram_tensor("sdiag", [128, p.n_win * p.win], edt,
                             kind="ExternalInput")
    xown_d = nc.dram_tensor("xown", [p.npc, F_IN], edt, kind="ExternalInput")
    w1_d = nc.dram_tensor("w1", [F_IN, H1D], f32, kind="ExternalInput")
    b1_d = nc.dram_tensor("b1", [H1D, 1], f32, kind="ExternalInput")
    w2_d = nc.dram_tensor("w2", [H1D, H2D], f32, kind="ExternalInput")
    b2_d = nc.dram_tensor("b2", [H2D, 1], f32, kind="ExternalInput")
    w3_d = nc.dram_tensor("w3", [H2D, H3D], f32, kind="ExternalInput")
    b3_d = nc.dram_tensor("b3", [H3D, 1], f32, kind="ExternalInput")
    w4_d = nc.dram_tensor("w4", [H3D, OUTD], f32, kind="ExternalInput")
    b4_d = nc.dram_tensor("b4", [OUTD, 1], f32, kind="ExternalInput")
    out_d = nc.dram_tensor("out", [p.npc, OUTD], f32, kind="ExternalOutput")

    h1_shard = nc.dram_tensor("h1_shard", [p.npc, H1D], edt)
    h1_full = nc.dram_tensor("h1_full", [p.n_nodes, H1D], edt,
                             addr_space="Shared")

    with tile.TileContext(nc) as tc:
        with (
            tc.tile_pool(name="const", bufs=1) as cpool,
            tc.tile_pool(name="gather", bufs=16) as gpool,
            tc.tile_pool(name="sel", bufs=3) as spool,
            tc.tile_pool(name="work", bufs=3) as wpool,
            tc.tile_pool(name="persist", bufs=1) as ppool,
            tc.tile_pool(name="psum", bufs=2, space="PSUM") as pspool,
            tc.tile_pool(name="psumfc", bufs=1, space="PSUM") as pfpool,
        ):
            # ---- constants / metadata to SBUF
            idx_s = cpool.tile([128, 2 * (p.S // 16)], i16)
            nc.sync.dma_start(idx_s[:], idx_d[:, :])
            sdiag_s = cpool.tile([128, p.n_win * p.win], edt)
            nc.sync.dma_start(sdiag_s[:], sdiag_d[:, :])
            w1_s = cpool.tile([F_IN, H1D], f32)
            nc.sync.dma_start(w1_s[:], w1_d[:, :])
            b1_s = cpool.tile([H1D, 1], f32)
            nc.sync.dma_start(b1_s[:], b1_d[:, :])
            w2_s = cpool.tile([H1D, H2D], f32)
            nc.sync.dma_start(w2_s[:], w2_d[:, :])
            b2_s = cpool.tile([H2D, 1], f32)
            nc.sync.dma_start(b2_s[:], b2_d[:, :])
            w3_s = cpool.tile([H2D, H3D], f32)
            nc.sync.dma_start(w3_s[:], w3_d[:, :])
            b3_s = cpool.tile([H3D, 1], f32)
            nc.sync.dma_start(b3_s[:], b3_d[:, :])
            w4_s = cpool.tile([H3D, OUTD], f32)
            nc.sync.dma_start(w4_s[:], w4_d[:, :])
            b4_s = cpool.tile([OUTD, 1], f32)
            nc.sync.dma_start(b4_s[:], b4_d[:, :])
            ident = cpool.tile([128, 128], f32)
            make_identity(nc, ident[:])

            h2T = ppool.tile([H2D, p.npc], f32)

            call_seq = [0]

            # ---------------- one GCN layer ----------------
            def gcn_layer(table_ap_fn, own_ap, w_s, b_s, out_feat, sink,
                          idx_base=0):
                issued = [0] * p.n_half
                bufs = {}

                def issue_next(hh):
                    ci = p.calls_h[hh][issued[hh]]
                    _, o, n = p.calls[ci]
                    gb = gpool.tile([128, (CALL // 128) * F_IN], edt, tag="gb")
                    out3 = gb[:].rearrange("q (t e) -> q t e", e=F_IN)
                    nc.gpsimd.dma_gather(
                        out_ap=out3[:, :n // 128, :],
                        in_ap=table_ap_fn(hh),
                        idxs_ap=idx_s[:, idx_base + o // 16:
                                      idx_base + (o + n) // 16],
                        num_idxs=n,
                        num_idxs_reg=n,
                        elem_size=F_IN,
                        single_packet=True,
                        queue_num=call_seq[0] % n_queues,
                    )
                    call_seq[0] += 1
                    bufs[ci] = gb
                    issued[hh] += 1

                def ensure(hh, local_needed, margin=5):
                    tgt = min(local_needed + margin, len(p.calls_h[hh]) - 1)
                    while issued[hh] <= tgt:
                        issue_next(hh)

                for ww in range(p.n_win):
                    for hh in range(p.n_half):
                        if p.win_need[ww][hh] >= 0:
                            ensure(hh, p.win_need[ww][hh])
                    q0, lst = p.win_ents[ww]
                    sbf = spool.tile([128, len(lst) * p.win], edt, tag="sbf")
                    nc.sync.dma_start(
                        sbf[:], smat_d[:, q0 * p.win:(q0 + len(lst)) * p.win])
                    wsz = p.win_sizes[ww]
                    xo = wpool.tile([128, F_IN], edt, tag="xo")
                    nc.sync.dma_start(
                        xo[:wsz, :], own_ap[ww * p.win: ww * p.win + wsz, :])
                    pag = pspool.tile([128, p.win], f32, tag="pag")
                    # self-loop term: x_own[window]^T @ diag(dinv^2)
                    nc.tensor.matmul(
                        pag[:], lhsT=xo[:wsz, :],
                        rhs=sdiag_s[:wsz, ww * p.win:(ww + 1) * p.win],
                        start=True, stop=(not lst))
                    for k, (hh, ci, tin) in enumerate(lst):
                        nc.tensor.matmul(
                            pag[:],
                            lhsT=bufs[ci][:, tin * F_IN:(tin + 1) * F_IN],
                            rhs=sbf[:, k * p.win:(k + 1) * p.win],
                            start=False, stop=(k == len(lst) - 1))
                    aggT = wpool.tile([128, p.win], f32, tag="aggT")
                    nc.vector.tensor_copy(aggT[:], pag[:])
                    ph = pspool.tile([out_feat, p.win], f32, tag="ph")
                    nc.tensor.matmul(ph[:], lhsT=w_s[:], rhs=aggT[:],
                                     start=True, stop=True)
                    sink(ww, wsz, ph, b_s)

            # ---- layer 1: x -> h1_shard (node-major, via PE transpose)
            def sink1(ww, wsz, ph, b_s):
                hT = wpool.tile([128, p.win], f32, tag="hT")
                nc.scalar.activation(hT[:], ph[:], AF.Tanh,
                                     bias=b_s[:, 0:1])
                pt = pspool.tile([128, 128], f32, tag="pt")
                nc.tensor.transpose(pt[:], hT[:], ident[:])
                hw_ = wpool.tile([128, 128], edt, tag="hw")
                nc.vector.tensor_copy(hw_[:], pt[:])
                nc.sync.dma_start(
                    h1_shard[ww * p.win: ww * p.win + wsz, :],
                    hw_[:wsz, :])

            gcn_layer(lambda hh: x_d[hh * p.half:
                                     min((hh + 1) * p.half, p.n_nodes), :],
                      xown_d, w1_s, b1_s, H1D, sink1)

            # ---- exchange h1 shards
            nc.gpsimd.collective_compute(
                "AllGather", mybir.AluOpType.bypass,
                replica_groups=[list(range(p.n_cores))],
                ins=[h1_shard[:, :]], outs=[h1_full[:, :]])

            # ---- layer 2: h1_full -> h2T (kept on-chip, feat-major)
            def sink2(ww, wsz, ph, b_s):
                nc.scalar.activation(
                    h2T[:, ww * p.win: ww * p.win + wsz],
                    ph[:, :wsz], AF.Tanh, bias=b_s[:, 0:1])

            gcn_layer(lambda hh: h1_full[hh * p.half:
                                         min((hh + 1) * p.half, p.n_nodes), :],
                      h1_shard, w2_s, b2_s, H2D, sink2,
                      idx_base=p.S // 16)

            # ---- fc layers on the dst shard
            for c0 in range(0, p.npc, NCHUNK):
                cs = min(NCHUNK, p.npc - c0)
                p3 = pfpool.tile([H3D, NCHUNK], f32, tag="p3")
                nc.tensor.matmul(p3[:, :cs], lhsT=w3_s[:],
                                 rhs=h2T[:, c0:c0 + cs],
                                 start=True, stop=True)
                h3 = wpool.tile([H3D, NCHUNK], f32, tag="h3")
                nc.scalar.activation(h3[:, :cs], p3[:, :cs], AF.Tanh,
                                     bias=b3_s[:, 0:1])
                p4 = pfpool.tile([OUTD, NCHUNK], f32, tag="p4")
                nc.tensor.matmul(p4[:, :cs], lhsT=w4_s[:], rhs=h3[:, :cs],
                                 start=True, stop=True)
                ob = wpool.tile([OUTD, NCHUNK], f32, tag="ob")
                nc.vector.tensor_scalar(
                    out=ob[:, :cs], in0=p4[:, :cs],
                    scalar1=b4_s[0:1, 0:1], scalar2=None, op0=OP.add)
                nc.sync.dma_start(out_d[c0:c0 + cs, :], ob[0:1, :cs])

    nc.compile()
    return nc


def make_in_maps(p, inputs, edge_dt="float16"):
    np_edt = dict(float32=np.float32, float16=np.float16)[edge_dt]
    x = np.ascontiguousarray(np.asarray(inputs["x"]).astype(np_edt))
    maps = []
    for c in range(p.n_cores):
        maps.append({
            "x": x,
            "midx": p.idx_arr[c],
            "smat": p.smat[c],
            "sdiag": p.sdiag[c],
            "xown": np.ascontiguousarray(
                x[p.perm[c * p.npc:(c + 1) * p.npc]]),
            "w1": np.asarray(inputs["W1"], dtype=np.float32),
            "b1": np.asarray(inputs["b1"], dtype=np.float32).reshape(-1, 1),
            "w2": np.asarray(inputs["W2"], dtype=np.float32),
            "b2": np.asarray(inputs["b2"], dtype=np.float32).reshape(-1, 1),
            "w3": np.asarray(inputs["W3"], dtype=np.float32),
            "b3": np.asarray(inputs["b3"], dtype=np.float32).reshape(-1, 1),
            "w4": np.asarray(inputs["W4"], dtype=np.float32),
            "b4": np.asarray(inputs["b4"], dtype=np.float32).reshape(-1, 1),
        })
    return maps


_CACHE = {}


def kernel(_trace=False, **inputs):
    from concourse.bass_utils import run_bass_kernel_spmd

    edge_index = np.asarray(inputs["edge_index"])
    p = make_plan(edge_index)
    key = (p.S, tuple(int(c[2]) for c in p.calls))
    if key not in _CACHE:
        _CACHE[key] = build_program(p)
    nc = _CACHE[key]
    res = run_bass_kernel_spmd(nc, make_in_maps(p, inputs),
                               core_ids=list(range(p.n_cores)),
                               trace=_trace)
    res_cat = np.concatenate(
        [res.results[c]["out"] for c in range(p.n_cores)], axis=0)
    out = np.empty_like(res_cat)
    out[p.perm] = res_cat
    if _trace:
        return out, res
    return out
